# revision 35
# baseline (speedup 1.0000x reference)
"""EqPBCNN (perturbation-based nonlinearity compensation NN) Trainium2 Bass kernel.

Data-parallel over 8 NeuronCores: batch 65536 -> 8192 per core.

Math (per sample, per polarization p):
  A~[h]   = sum_q x[n_h,q] * conj(x[m_h+n_h,q])        (pol-summed, same for both p)
  F[h,p]  = SYM[h] * A~[h] * x[m_h,p]
  h1 = CLrelu(F @ W1^T); h2 = CLrelu(h1 @ W2^T); E = h2 @ W3^T
  out = x[center,p] + E * 10^(task0/10)/2

Kernel pipeline per chunk of nb=512 samples (taps on partitions, samples on
free dim, complex handled as [re|im] halves of the free dim, bf16 data
shipped pre-converted and chunk-contiguous from the host):
  stage1: gather MMs (PE, one-hot SEL) -> f32 psum pairs [A|C]x[re|im]
          pair evacuations (ACT/DVE)   -> bf16 SBUF acat=[ar|ai], ccat=[cr|ci]
          products (DVE/Pool bf16)     -> t01 = acat*ccat = [ar*cr | ai*ci]
                                          t23 = cross-sliced [ar*ci | ai*cr]
  stage2: R matmuls (PE)               -> Rre then Rim (1 psum bank each),
                                          accumulated straight from t01/t23 with
                                          sign-folded weights (G adds absorbed)
          xm gather + T products       -> u01/u23 = xmcat * rcat
          fin matmuls (PE)             -> h1 (T adds absorbed via +/- FIN blocks)
          per-chunk tail               -> lrelu/W2/lrelu/W3/exp/residual add

Measured: 232 us on silicon (NTFF exec_time_ns, max of 8 cores), rel err 4.6e-3.
"""
import numpy as np

# ---------------- problem constants (hardcoded; must match reference) -------
BATCH = 65536
MT, LH = 41, 20          # filter taps, half window
NM = 2                   # modes / polarizations
H1, H2 = 2, 10
SLOPE = 0.01
NCORES = 8
BCORE = BATCH // NCORES  # 8192
NB = 512                 # samples per chunk
NB2 = 2 * NB             # free dim of [re|im] stacked tiles
NCHUNK = BCORE // NB     # 16
GRP = 4                  # chunks per tail group
NGRP = NCHUNK // GRP
ROWS = MT * NM           # 82 = tap*2 + mode

_idx = [(m, n) for m in range(-LH, LH + 1) for n in range(-LH, LH + 1)
        if abs(m * n) <= LH and abs(m + n) <= LH and n >= m]
H = len(_idx)            # 175
M_ARR = np.array([t[0] for t in _idx], np.int32)
N_ARR = np.array([t[1] for t in _idx], np.int32)
A_TAP = N_ARR + LH           # source tap for En
C_TAP = M_ARR + N_ARR + LH   # source tap for Emn (conjugated side)
SYM = np.where(M_ARR != N_ARR, 2.0, 1.0).astype(np.float32)
M_VALS = sorted(set(M_ARR.tolist()))     # 25 distinct m values
NMV = len(M_VALS)
M_POS = {m: i for i, m in enumerate(M_VALS)}
NO = H1 * NMV * NM       # 100 rows of R space: (o, mi, p)
NSTACK = 2 * H           # 350 rows: (h, pol)
KSPLITS = [(0, 128), (128, 128), (256, NSTACK - 256)]   # psplits of the stacks


def _orow(o, mi, p):
    return (o * NMV + mi) * NM + p


def _hrow(p, o, comp):
    return (p * H1 + o) * 2 + comp


def _h2row(p, q, comp):
    return (p * H2 + q) * 2 + comp


def build_static():
    """Weight-independent constant matrices (stored f32, cast to bf16 on chip)."""
    # gather selections: stack row r = 2h+q reads x row 2*tap+q
    SELA = np.zeros((ROWS, NSTACK), np.float32)
    SELC = np.zeros((ROWS, NSTACK), np.float32)
    for h in range(H):
        for q in range(NM):
            r = 2 * h + q
            SELA[2 * A_TAP[h] + q, r] = 1.0
            SELC[2 * C_TAP[h] + q, r] = 1.0
    # xm gather: row (o,mi,p) reads tap m
    XREPW = np.zeros((ROWS, NO), np.float32)
    for o in range(H1):
        for mi, mv in enumerate(M_VALS):
            for p in range(NM):
                XREPW[2 * (mv + LH) + p, _orow(o, mi, p)] = 1.0
    # fin contraction [100, 32]: blocks for u01_lo, u01_hi, u23_lo, u23_hi
    #   T_re = u01_lo - u01_hi  -> comp 0 rows ;  T_im = u23_lo + u23_hi -> comp 1
    # FINC [100, 4*32]: term block j (32 cols, zero-padded so the matmul
    # initializes the full 32-row strip) for u01_lo, u01_hi, u23_lo, u23_hi
    FINC = np.zeros((NO, 128), np.float32)
    for o in range(H1):
        for mi in range(NMV):
            for p in range(NM):
                r = _orow(o, mi, p)
                FINC[r, 0 * 32 + _hrow(p, o, 0)] = 1.0
                FINC[r, 1 * 32 + _hrow(p, o, 0)] = -1.0
                FINC[r, 2 * 32 + _hrow(p, o, 1)] = 1.0
                FINC[r, 3 * 32 + _hrow(p, o, 1)] = 1.0
    return {"SELA": SELA, "SELC": SELC, "XREPW": XREPW, "FINC": FINC}


def fold_weights(W1r, W1i, W2r, W2i, W3r, W3i):
    """Runtime weight folding into matmul lhsT constants.

    RW2[k] : [128, 4*NO] psplit-k lhsT, col blocks [Wr, Wi, -Wr, -Wi]
    (rows (h,q)-interleaved; q rows share the weight = pol pre-sum).
    R accumulation (G adds absorbed):
      t01 = [ar*cr | ai*ci]   t23_a = ar*ci   t23_b = ai*cr
      G_re = t01_lo + t01_hi      G_im = t23_b - t23_a
      Rre  = Wr*G_re - Wi*G_im = Wr*t01_lo + Wr*t01_hi + Wi*t23_a - Wi*t23_b
      Rim  = Wi*G_re + Wr*G_im = Wi*t01_lo + Wi*t01_hi - Wr*t23_a + Wr*t23_b
    """
    Wr = (W1r * SYM[None, None, :]).astype(np.float32)   # [p, o, h]
    Wi = (W1i * SYM[None, None, :]).astype(np.float32)
    RWr = np.zeros((NSTACK, NO), np.float32)
    RWi = np.zeros((NSTACK, NO), np.float32)
    for h in range(H):
        mi = M_POS[M_ARR[h]]
        for p in range(NM):          # output pol (weights are per-pol)
            for q in range(NM):      # source stack row pol (pre-summed)
                r = 2 * h + q
                for o in range(H1):
                    c = _orow(o, mi, p)
                    RWr[r, c] += Wr[p, o, h]
                    RWi[r, c] += Wi[p, o, h]
    RW2 = np.zeros((3, 128, 4 * NO), np.float32)
    for k, (r0, rk) in enumerate(KSPLITS):
        RW2[k, :rk, 0 * NO:1 * NO] = RWr[r0:r0 + rk]
        RW2[k, :rk, 1 * NO:2 * NO] = RWi[r0:r0 + rk]
        RW2[k, :rk, 2 * NO:3 * NO] = -RWr[r0:r0 + rk]
        RW2[k, :rk, 3 * NO:4 * NO] = -RWi[r0:r0 + rk]
    # W2 lhsT [8, 40]
    W2L = np.zeros((8, 2 * H2 * NM), np.float32)
    for p in range(NM):
        for q in range(H2):
            for o in range(H1):
                W2L[_hrow(p, o, 0), _h2row(p, q, 0)] += W2r[p, q, o]
                W2L[_hrow(p, o, 1), _h2row(p, q, 0)] -= W2i[p, q, o]
                W2L[_hrow(p, o, 0), _h2row(p, q, 1)] += W2i[p, q, o]
                W2L[_hrow(p, o, 1), _h2row(p, q, 1)] += W2r[p, q, o]
    # replicate W2L at the 4 strip offsets (lhsT base must match rhs base)
    W2L4 = np.zeros((128, 2 * H2 * NM), np.float32)
    for cg in range(4):
        W2L4[32 * cg:32 * cg + 8] = W2L
    W2L = W2L4
    # W3 lhsT [40, 4]: out rows [re_p0, re_p1, im_p0, im_p1]; 1/NM folded
    W3L = np.zeros((2 * H2 * NM, 4), np.float32)
    s = 1.0 / NM
    for p in range(NM):
        for q in range(H2):
            W3L[_h2row(p, q, 0), 0 + p] += W3r[p, 0, q] * s
            W3L[_h2row(p, q, 1), 0 + p] -= W3i[p, 0, q] * s
            W3L[_h2row(p, q, 0), 2 + p] += W3i[p, 0, q] * s
            W3L[_h2row(p, q, 1), 2 + p] += W3r[p, 0, q] * s
    return {"RW2": RW2, "W2L": W2L, "W3L": W3L}


# ---------------------------------------------------------------------------
def build_nc(bcore=BCORE, nb=NB, lrelu_mode="act"):
    """Build the Bass program for one core processing `bcore` samples."""
    import concourse.bass as bass
    import concourse.bacc as bacc
    import concourse.mybir as mybir
    from concourse.tile import TileContext
    import bass_rust

    nchunk = bcore // nb
    assert nchunk * nb == bcore
    grp = GRP if nchunk % GRP == 0 else 1
    ngrp = nchunk // grp
    nb2 = 2 * nb
    f32 = mybir.dt.float32
    bf16 = mybir.dt.bfloat16
    AF = bass_rust.ActivationFunctionType
    OP = mybir.AluOpType

    nc = bacc.Bacc(None, target_bir_lowering=False, debug=False)
    xTb = nc.declare_dram_parameter("xTb", [ngrp, ROWS, grp * nb2], bf16, isOutput=False)
    xcen = nc.declare_dram_parameter("xcen", [nchunk, 8, nb], f32, isOutput=False)
    selaD = nc.declare_dram_parameter("SELA", [ROWS, NSTACK], f32, isOutput=False)
    selcD = nc.declare_dram_parameter("SELC", [ROWS, NSTACK], f32, isOutput=False)
    xrwD = nc.declare_dram_parameter("XREPW", [ROWS, NO], f32, isOutput=False)
    finD = nc.declare_dram_parameter("FINC", [NO, 128], f32, isOutput=False)
    rwD = nc.declare_dram_parameter("RW2", [3, 128, 4 * NO], f32, isOutput=False)
    w2D = nc.declare_dram_parameter("W2L", [128, 40], f32, isOutput=False)
    w3D = nc.declare_dram_parameter("W3L", [40, 4], f32, isOutput=False)
    outs_d = [nc.declare_dram_parameter(f"out{j}", [4 * grp, nb], f32, isOutput=True)
              for j in range(ngrp)]

    with TileContext(nc) as tc:
        with (
            tc.tile_pool(name="consts", bufs=1) as cp,
            tc.tile_pool(name="xt", bufs=3) as xp,
            tc.tile_pool(name="ev", bufs=2) as ep_,      # evacuated stacks
            tc.tile_pool(name="pr", bufs=2) as prp,      # products
            tc.tile_pool(name="tt", bufs=2) as ttp,      # T tiles
            tc.tile_pool(name="small", bufs=2) as sp,
            tc.tile_pool(name="gath", bufs=3, space="PSUM") as gp,   # gather pair psums
            tc.tile_pool(name="racc", bufs=1, space="PSUM") as rp,   # R halves
            tc.tile_pool(name="tailp", bufs=1, space="PSUM") as tp_,  # shared h1/h2
        ):
            def lrelu(dst, src):
                if lrelu_mode == "act":
                    nc.scalar.activation(dst, src, AF.Lrelu, alpha=SLOPE)
                else:
                    nc.vector.tensor_scalar_mul(dst, src, SLOPE)
                    nc.vector.tensor_tensor(dst, dst, src, op=OP.max)

            def const_tile(src_ap, name):
                t32 = cp.tile(list(src_ap.shape), f32, name=name + "_32")
                nc.gpsimd.dma_start(out=t32[:], in_=src_ap)
                tr = cp.tile(list(src_ap.shape), bf16, name=name)
                nc.vector.tensor_copy(tr[:], t32[:])
                return tr

            sela = const_tile(selaD[:], "sela")
            selc = const_tile(selcD[:], "selc")
            xrw = const_tile(xrwD[:], "xrw")
            fin = const_tile(finD[:], "fin")
            rw = [const_tile(rwD[k], f"rw{k}") for k in range(3)]
            w2_sb = const_tile(w2D[:], "w2")
            w3_sb = const_tile(w3D[:], "w3")

            def finish(fin):
                """Deferred chunk tail: W2/W3 MLP, E*P multiply, residual add,
                store. Emitted mid-next-chunk so these late-dependency ops do
                not head-of-line-block the PE/DVE/Pool FIFOs."""
                p_h2 = tp_.tile([40, nb], f32, tag="hx")
                nc.tensor.matmul(p_h2[:], w2_sb[0:8], fin["h1s"][:],
                                 start=True, stop=True)
                h2s = sp.tile([40, nb], bf16, tag="h2s", bufs=3)
                lrelu(h2s[:], p_h2[:])
                p_E = tp_.tile([40, nb], f32, tag="hx")
                nc.tensor.matmul(p_E[0:4], w3_sb[:], h2s[:], start=True, stop=True)
                ept_c = sp.tile([4, nb], f32, tag="ept_c", bufs=3)
                nc.vector.tensor_tensor(ept_c[:], p_E[0:4], fin["pex"][:],
                                        op=OP.mult)
                o_c = sp.tile([4, nb], f32, tag="o_c", bufs=3)
                nc.gpsimd.tensor_tensor(o_c[:], ept_c[:], fin["cen"][:], op=OP.add)
                nc.sync.dma_start(out=outs_d[fin["g"]][4 * fin["cg"]:4 * fin["cg"] + 4],
                                  in_=o_c[:])

            def stage1(g, cg, xc4, pending):
                """Gathers + evacuations + products (+ xm gather/evac) for one chunk."""
                xre = xc4[:, cg * nb2:cg * nb2 + nb]
                xim = xc4[:, cg * nb2 + nb:cg * nb2 + nb2]
                pk = []
                for k, (r0, rk) in enumerate(KSPLITS):
                    a_sl = sela[:, r0:r0 + rk]
                    c_sl = selc[:, r0:r0 + rk]
                    p_c = gp.tile([128, nb2], f32, tag="pg")
                    p_a = gp.tile([128, nb2], f32, tag="pg")
                    nc.tensor.matmul(p_c[:rk, 0:nb], c_sl, xre, start=True, stop=True)
                    nc.tensor.matmul(p_c[:rk, nb:nb2], c_sl, xim, start=True, stop=True)
                    nc.tensor.matmul(p_a[:rk, 0:nb], a_sl, xre, start=True, stop=True)
                    nc.tensor.matmul(p_a[:rk, nb:nb2], a_sl, xim, start=True, stop=True)
                    acat = ep_.tile([128, nb2], bf16, tag="acat", bufs=4)
                    ccat = ep_.tile([128, nb2], bf16, tag="ccat", bufs=4)
                    nc.scalar.copy(ccat[:rk], p_c[:rk])
                    if k == 0:
                        nc.scalar.copy(acat[:rk], p_a[:rk])
                    else:
                        nc.vector.tensor_copy(acat[:rk], p_a[:rk])
                    t01 = prp.tile([128, nb2], bf16, tag="t01", bufs=7)
                    t23 = prp.tile([128, nb2], bf16, tag="t23", bufs=7)
                    nc.vector.tensor_tensor(t01[:rk], acat[:rk], ccat[:rk], op=OP.mult)
                    if k == 2:
                        nc.vector.tensor_tensor(t23[:rk, 0:nb], acat[:rk, 0:nb],
                                                ccat[:rk, nb:nb2], op=OP.mult)
                        nc.vector.tensor_tensor(t23[:rk, nb:nb2], acat[:rk, nb:nb2],
                                                ccat[:rk, 0:nb], op=OP.mult)
                    else:
                        nc.gpsimd.tensor_tensor(t23[:rk, 0:nb], acat[:rk, 0:nb],
                                                ccat[:rk, nb:nb2], op=OP.mult)
                        nc.gpsimd.tensor_tensor(t23[:rk, nb:nb2], acat[:rk, nb:nb2],
                                                ccat[:rk, 0:nb], op=OP.mult)
                    pk.append((rw[k], rk, t01, t23))
                    if k == 1 and pending is not None:
                        finish(pending)
                return {"pk": pk, "xre": xre, "xim": xim,
                        "g": g, "cg": cg, "c": g * grp + cg}

            def stage2(st):
                """R accumulation + xm gather + T products + fin + tail."""
                pk = st["pk"]
                g, cg, c = st["g"], st["cg"], st["c"]
                rcat = ttp.tile([NO, nb2], bf16, tag="rcat", bufs=3)
                for half in range(2):
                    p_rh = rp.tile([NO, nb], f32, tag="rh")
                    for k3, (rwk, rk, t01, t23) in enumerate(pk):
                        st_ = (k3 == 0)
                        sp_ = (k3 == 2)
                        wr_ = rwk[:rk, 0 * NO:1 * NO]
                        wi_ = rwk[:rk, 1 * NO:2 * NO]
                        wrn = rwk[:rk, 2 * NO:3 * NO]
                        win = rwk[:rk, 3 * NO:4 * NO]
                        if half == 0:
                            nc.tensor.matmul(p_rh[:], wr_, t01[:rk, 0:nb], start=st_, stop=False)
                            nc.tensor.matmul(p_rh[:], wr_, t01[:rk, nb:nb2], start=False, stop=False)
                            nc.tensor.matmul(p_rh[:], wi_, t23[:rk, 0:nb], start=False, stop=False)
                            nc.tensor.matmul(p_rh[:], win, t23[:rk, nb:nb2], start=False, stop=sp_)
                        else:
                            nc.tensor.matmul(p_rh[:], wi_, t01[:rk, 0:nb], start=st_, stop=False)
                            nc.tensor.matmul(p_rh[:], wi_, t01[:rk, nb:nb2], start=False, stop=False)
                            nc.tensor.matmul(p_rh[:], wrn, t23[:rk, 0:nb], start=False, stop=False)
                            nc.tensor.matmul(p_rh[:], wr_, t23[:rk, nb:nb2], start=False, stop=sp_)
                    nc.scalar.copy(rcat[:, half * nb:half * nb + nb], p_rh[:])
                p_xm = gp.tile([128, nb2], f32, tag="pg")
                nc.tensor.matmul(p_xm[:NO, 0:nb], xrw[:], st["xre"], start=True, stop=True)
                nc.tensor.matmul(p_xm[:NO, nb:nb2], xrw[:], st["xim"], start=True, stop=True)
                xmcat = ttp.tile([NO, nb2], bf16, tag="xmcat", bufs=3)
                nc.vector.tensor_copy(xmcat[:], p_xm[:NO])
                u01 = ttp.tile([NO, nb2], bf16, tag="u01", bufs=3)
                u23 = ttp.tile([NO, nb2], bf16, tag="u23", bufs=3)
                nc.vector.tensor_tensor(u01[:], xmcat[:], rcat[:], op=OP.mult)
                nc.vector.tensor_tensor(u23[:, 0:nb], xmcat[:, 0:nb],
                                        rcat[:, nb:nb2], op=OP.mult)
                nc.gpsimd.tensor_tensor(u23[:, nb:nb2], xmcat[:, nb:nb2],
                                        rcat[:, 0:nb], op=OP.mult)
                p_h1 = tp_.tile([40, nb], f32, tag="hx")
                nc.tensor.matmul(p_h1[0:32], fin[:, 0:32], u01[:, 0:nb], start=True, stop=False)
                nc.tensor.matmul(p_h1[0:32], fin[:, 32:64], u01[:, nb:nb2], start=False, stop=False)
                nc.tensor.matmul(p_h1[0:32], fin[:, 64:96], u23[:, 0:nb], start=False, stop=False)
                nc.tensor.matmul(p_h1[0:32], fin[:, 96:128], u23[:, nb:nb2], start=False, stop=True)
                h1s = sp.tile([8, nb], bf16, tag="h1s", bufs=3)
                lrelu(h1s[:], p_h1[0:8])
                cen_c = sp.tile([4, nb], f32, tag="cen_c", bufs=3)
                tsk_c = sp.tile([4, nb], f32, tag="tsk_c", bufs=3)
                nc.sync.dma_start(out=cen_c[:], in_=xcen[c, 0:4])
                nc.sync.dma_start(out=tsk_c[:], in_=xcen[c, 4:8])
                pex_c = sp.tile([4, nb], f32, tag="pex_c", bufs=3)
                nc.scalar.activation(pex_c[:], tsk_c[:], AF.Exp,
                                     scale=float(np.log(10.0) / 10.0))
                return {"h1s": h1s, "pex": pex_c, "cen": cen_c, "g": g, "cg": cg}

            pending = None
            for g in range(ngrp):
                xc4 = xp.tile([ROWS, grp * nb2], bf16, tag="xc4", bufs=2)
                nc.sync.dma_start(out=xc4[:], in_=xTb[g])
                for cg in range(grp):
                    pending = stage2(stage1(g, cg, xc4, pending))
            finish(pending)
    nc.compile()
    return nc


def _prep_core_inputs(inputs, static, folded):
    """Shard + lay out inputs per core. Returns list of in_maps."""
    import ml_dtypes
    bf16 = ml_dtypes.bfloat16
    xr = np.ascontiguousarray(inputs["x_real"]).reshape(BATCH, ROWS)
    xi = np.ascontiguousarray(inputs["x_imag"]).reshape(BATCH, ROWS)
    t0 = np.ascontiguousarray(inputs["task_info"][:, 0])
    shared = {
        "SELA": static["SELA"], "SELC": static["SELC"], "XREPW": static["XREPW"],
        "FINC": static["FINC"], "RW2": folded["RW2"], "W2L": folded["W2L"],
        "W3L": folded["W3L"],
    }
    in_maps = []
    for cc in range(NCORES):
        s = slice(cc * BCORE, (cc + 1) * BCORE)
        m = dict(shared)
        # xTb [nchunk, ROWS, 2*NB] bf16: [c, r, 0:NB]=re, [c, r, NB:]=im
        xrc = xr[s].reshape(NCHUNK, NB, ROWS).transpose(0, 2, 1)
        xic = xi[s].reshape(NCHUNK, NB, ROWS).transpose(0, 2, 1)
        xtb = np.empty((NCHUNK, ROWS, NB2), bf16)
        xtb[:, :, 0:NB] = xrc.astype(bf16)
        xtb[:, :, NB:NB2] = xic.astype(bf16)
        m["xTb"] = xtb
        # xcen [nchunk, 8, NB] f32: rows 0-3 center [re_p0, re_p1, im_p0, im_p1],
        # rows 4-7 task t0 replicated
        cen = np.empty((NCHUNK, 8, NB), np.float32)
        cen[:, 0] = xrc[:, 2 * LH + 0]
        cen[:, 1] = xrc[:, 2 * LH + 1]
        cen[:, 2] = xic[:, 2 * LH + 0]
        cen[:, 3] = xic[:, 2 * LH + 1]
        cen[:, 4:8] = t0[s].reshape(NCHUNK, 1, NB)
        m["xcen"] = cen
        in_maps.append(m)
    return in_maps


_CACHE = {}


def kernel(**inputs):
    from concourse.bass_utils import run_bass_kernel_spmd

    static = build_static()
    folded = fold_weights(
        np.asarray(inputs["W1_real"]), np.asarray(inputs["W1_imag"]),
        np.asarray(inputs["W2_real"]), np.asarray(inputs["W2_imag"]),
        np.asarray(inputs["W3_real"]), np.asarray(inputs["W3_imag"]),
    )
    if "nc" not in _CACHE:
        _CACHE["nc"] = build_nc()
    nc = _CACHE["nc"]
    in_maps = _prep_core_inputs(inputs, static, folded)
    res = run_bass_kernel_spmd(nc, in_maps, list(range(NCORES)))
    out = np.empty((BATCH, NM, 2), np.float32)
    for cc in range(NCORES):
        s0 = cc * BCORE
        for g in range(NGRP):
            o16 = res.results[cc][f"out{g}"]        # [16, NB]
            for cg in range(GRP):
                c = g * GRP + cg
                sl = slice(s0 + c * NB, s0 + (c + 1) * NB)
                out[sl, 0, 0] = o16[4 * cg + 0]
                out[sl, 1, 0] = o16[4 * cg + 1]
                out[sl, 0, 1] = o16[4 * cg + 2]
                out[sl, 1, 1] = o16[4 * cg + 3]
    return out


# revision 36
# speedup vs baseline: 1.1804x; 1.1804x over previous
"""EqPBCNN (perturbation-based nonlinearity compensation NN) Trainium2 Bass kernel.

Data-parallel over 8 NeuronCores: batch 65536 -> 8192 per core.

Math (per sample, per polarization p):
  A~[h]   = sum_q x[n_h,q] * conj(x[m_h+n_h,q])        (pol-summed, same for both p)
  F[h,p]  = SYM[h] * A~[h] * x[m_h,p]
  h1 = CLrelu(F @ W1^T); h2 = CLrelu(h1 @ W2^T); E = h2 @ W3^T
  out = x[center,p] + E * 10^(task0/10)/2

Kernel pipeline per chunk of nb=512 samples (taps on partitions, samples on
free dim, complex handled as [re|im] halves of the free dim, bf16 data
shipped pre-converted and chunk-contiguous from the host):
  stage1: gather MMs (PE, one-hot SEL) -> f32 psum pairs [A|C]x[re|im]
          pair evacuations (ACT/DVE)   -> bf16 SBUF acat=[ar|ai], ccat=[cr|ci]
          products (DVE/Pool bf16)     -> t01 = acat*ccat = [ar*cr | ai*ci]
                                          t23 = cross-sliced [ar*ci | ai*cr]
  stage2: R matmuls (PE)               -> Rre then Rim (1 psum bank each),
                                          accumulated straight from t01/t23 with
                                          sign-folded weights (G adds absorbed)
          xm gather + T products       -> u01/u23 = xmcat * rcat
          fin matmuls (PE)             -> h1 (T adds absorbed via +/- FIN blocks)
          per-chunk tail               -> lrelu/W2/lrelu/W3/exp/residual add

Measured: 232 us on silicon (NTFF exec_time_ns, max of 8 cores), rel err 4.6e-3.
"""
import numpy as np

# ---------------- problem constants (hardcoded; must match reference) -------
BATCH = 65536
MT, LH = 41, 20          # filter taps, half window
NM = 2                   # modes / polarizations
H1, H2 = 2, 10
SLOPE = 0.01
NCORES = 8
BCORE = BATCH // NCORES  # 8192
NB = 512                 # samples per chunk
NB2 = 2 * NB             # free dim of [re|im] stacked tiles
NCHUNK = BCORE // NB     # 16
GRP = 4                  # chunks per tail group
NGRP = NCHUNK // GRP
ROWS = MT * NM           # 82 = tap*2 + mode

_idx = [(m, n) for m in range(-LH, LH + 1) for n in range(-LH, LH + 1)
        if abs(m * n) <= LH and abs(m + n) <= LH and n >= m]
H = len(_idx)            # 175
M_ARR = np.array([t[0] for t in _idx], np.int32)
N_ARR = np.array([t[1] for t in _idx], np.int32)
A_TAP = N_ARR + LH           # source tap for En
C_TAP = M_ARR + N_ARR + LH   # source tap for Emn (conjugated side)
SYM = np.where(M_ARR != N_ARR, 2.0, 1.0).astype(np.float32)
M_VALS = sorted(set(M_ARR.tolist()))     # 25 distinct m values
NMV = len(M_VALS)
M_POS = {m: i for i, m in enumerate(M_VALS)}
NO = H1 * NMV * NM       # 100 rows of R space: (o, mi, p)
NSTACK = 2 * H           # 350 rows: (h, pol)
KSPLITS = [(0, 128), (128, 128), (256, NSTACK - 256)]   # psplits of the stacks


def _orow(o, mi, p):
    return (o * NMV + mi) * NM + p


def _hrow(p, o, comp):
    return (p * H1 + o) * 2 + comp


def _h2row(p, q, comp):
    return (p * H2 + q) * 2 + comp


def build_static():
    """Weight-independent constant matrices (stored f32, cast to bf16 on chip)."""
    # gather selections: stack row r = 2h+q reads x row 2*tap+q
    SELA = np.zeros((ROWS, NSTACK), np.float32)
    SELC = np.zeros((ROWS, NSTACK), np.float32)
    for h in range(H):
        for q in range(NM):
            r = 2 * h + q
            SELA[2 * A_TAP[h] + q, r] = 1.0
            SELC[2 * C_TAP[h] + q, r] = 1.0
    # xm gather: row (o,mi,p) reads tap m
    XREPW = np.zeros((ROWS, NO), np.float32)
    for o in range(H1):
        for mi, mv in enumerate(M_VALS):
            for p in range(NM):
                XREPW[2 * (mv + LH) + p, _orow(o, mi, p)] = 1.0
    # fin contraction [100, 32]: blocks for u01_lo, u01_hi, u23_lo, u23_hi
    #   T_re = u01_lo - u01_hi  -> comp 0 rows ;  T_im = u23_lo + u23_hi -> comp 1
    # FINC [100, 4*32]: term block j (32 cols, zero-padded so the matmul
    # initializes the full 32-row strip) for u01_lo, u01_hi, u23_lo, u23_hi
    FINC = np.zeros((NO, 128), np.float32)
    for o in range(H1):
        for mi in range(NMV):
            for p in range(NM):
                r = _orow(o, mi, p)
                FINC[r, 0 * 32 + _hrow(p, o, 0)] = 1.0
                FINC[r, 1 * 32 + _hrow(p, o, 0)] = -1.0
                FINC[r, 2 * 32 + _hrow(p, o, 1)] = 1.0
                FINC[r, 3 * 32 + _hrow(p, o, 1)] = 1.0
    return {"SELA": SELA, "SELC": SELC, "XREPW": XREPW, "FINC": FINC}


def fold_weights(W1r, W1i, W2r, W2i, W3r, W3i):
    """Runtime weight folding into matmul lhsT constants.

    RW2[k] : [128, 4*NO] psplit-k lhsT, col blocks [Wr, Wi, -Wr, -Wi]
    (rows (h,q)-interleaved; q rows share the weight = pol pre-sum).
    R accumulation (G adds absorbed):
      t01 = [ar*cr | ai*ci]   t23_a = ar*ci   t23_b = ai*cr
      G_re = t01_lo + t01_hi      G_im = t23_b - t23_a
      Rre  = Wr*G_re - Wi*G_im = Wr*t01_lo + Wr*t01_hi + Wi*t23_a - Wi*t23_b
      Rim  = Wi*G_re + Wr*G_im = Wi*t01_lo + Wi*t01_hi - Wr*t23_a + Wr*t23_b
    """
    Wr = (W1r * SYM[None, None, :]).astype(np.float32)   # [p, o, h]
    Wi = (W1i * SYM[None, None, :]).astype(np.float32)
    RWr = np.zeros((NSTACK, NO), np.float32)
    RWi = np.zeros((NSTACK, NO), np.float32)
    for h in range(H):
        mi = M_POS[M_ARR[h]]
        for p in range(NM):          # output pol (weights are per-pol)
            for q in range(NM):      # source stack row pol (pre-summed)
                r = 2 * h + q
                for o in range(H1):
                    c = _orow(o, mi, p)
                    RWr[r, c] += Wr[p, o, h]
                    RWi[r, c] += Wi[p, o, h]
    RW2 = np.zeros((3, 128, 4 * NO), np.float32)
    for k, (r0, rk) in enumerate(KSPLITS):
        RW2[k, :rk, 0 * NO:1 * NO] = RWr[r0:r0 + rk]
        RW2[k, :rk, 1 * NO:2 * NO] = RWi[r0:r0 + rk]
        RW2[k, :rk, 2 * NO:3 * NO] = -RWr[r0:r0 + rk]
        RW2[k, :rk, 3 * NO:4 * NO] = -RWi[r0:r0 + rk]
    # W2 lhsT [8, 40]
    W2L = np.zeros((8, 2 * H2 * NM), np.float32)
    for p in range(NM):
        for q in range(H2):
            for o in range(H1):
                W2L[_hrow(p, o, 0), _h2row(p, q, 0)] += W2r[p, q, o]
                W2L[_hrow(p, o, 1), _h2row(p, q, 0)] -= W2i[p, q, o]
                W2L[_hrow(p, o, 0), _h2row(p, q, 1)] += W2i[p, q, o]
                W2L[_hrow(p, o, 1), _h2row(p, q, 1)] += W2r[p, q, o]
    # replicate W2L at the 4 strip offsets (lhsT base must match rhs base)
    W2L4 = np.zeros((128, 2 * H2 * NM), np.float32)
    for cg in range(4):
        W2L4[32 * cg:32 * cg + 8] = W2L
    W2L = W2L4
    # W3 lhsT [40, 4]: out rows [re_p0, re_p1, im_p0, im_p1]; 1/NM folded
    W3L = np.zeros((2 * H2 * NM, 4), np.float32)
    s = 1.0 / NM
    for p in range(NM):
        for q in range(H2):
            W3L[_h2row(p, q, 0), 0 + p] += W3r[p, 0, q] * s
            W3L[_h2row(p, q, 1), 0 + p] -= W3i[p, 0, q] * s
            W3L[_h2row(p, q, 0), 2 + p] += W3i[p, 0, q] * s
            W3L[_h2row(p, q, 1), 2 + p] += W3r[p, 0, q] * s
    return {"RW2": RW2, "W2L": W2L, "W3L": W3L}


# ---------------------------------------------------------------------------
def build_nc(bcore=BCORE, nb=NB, lrelu_mode="act"):
    """Build the Bass program for one core processing `bcore` samples."""
    import concourse.bass as bass
    import concourse.bacc as bacc
    import concourse.mybir as mybir
    from concourse.tile import TileContext
    import bass_rust

    nchunk = bcore // nb
    assert nchunk * nb == bcore
    grp = GRP if nchunk % GRP == 0 else 1
    ngrp = nchunk // grp
    nb2 = 2 * nb
    f32 = mybir.dt.float32
    bf16 = mybir.dt.bfloat16
    AF = bass_rust.ActivationFunctionType
    OP = mybir.AluOpType

    nc = bacc.Bacc(None, target_bir_lowering=False, debug=False)
    xTb = nc.declare_dram_parameter("xTb", [ngrp, ROWS, grp * nb2], bf16, isOutput=False)
    xcen = nc.declare_dram_parameter("xcen", [nchunk, 8, nb], f32, isOutput=False)
    selaD = nc.declare_dram_parameter("SELA", [ROWS, NSTACK], f32, isOutput=False)
    selcD = nc.declare_dram_parameter("SELC", [ROWS, NSTACK], f32, isOutput=False)
    xrwD = nc.declare_dram_parameter("XREPW", [ROWS, NO], f32, isOutput=False)
    finD = nc.declare_dram_parameter("FINC", [NO, 128], f32, isOutput=False)
    rwD = nc.declare_dram_parameter("RW2", [3, 128, 4 * NO], f32, isOutput=False)
    w2D = nc.declare_dram_parameter("W2L", [128, 40], f32, isOutput=False)
    w3D = nc.declare_dram_parameter("W3L", [40, 4], f32, isOutput=False)
    outs_d = [nc.declare_dram_parameter(f"out{j}", [4 * grp, nb], f32, isOutput=True)
              for j in range(ngrp)]

    with TileContext(nc) as tc:
        with (
            tc.tile_pool(name="consts", bufs=1) as cp,
            tc.tile_pool(name="xt", bufs=3) as xp,
            tc.tile_pool(name="ev", bufs=2) as ep_,      # evacuated stacks
            tc.tile_pool(name="pr", bufs=2) as prp,      # products
            tc.tile_pool(name="tt", bufs=2) as ttp,      # T tiles
            tc.tile_pool(name="small", bufs=2) as sp,
            tc.tile_pool(name="gath", bufs=3, space="PSUM") as gp,   # gather pair psums
            tc.tile_pool(name="racc", bufs=1, space="PSUM") as rp,   # R halves
            tc.tile_pool(name="tailp", bufs=1, space="PSUM") as tp_,  # shared h1/h2
        ):
            def lrelu(dst, src):
                if lrelu_mode == "act":
                    nc.scalar.activation(dst, src, AF.Lrelu, alpha=SLOPE)
                else:
                    nc.vector.tensor_scalar_mul(dst, src, SLOPE)
                    nc.vector.tensor_tensor(dst, dst, src, op=OP.max)

            def const_tile(src_ap, name):
                t32 = cp.tile(list(src_ap.shape), f32, name=name + "_32")
                nc.gpsimd.dma_start(out=t32[:], in_=src_ap)
                tr = cp.tile(list(src_ap.shape), bf16, name=name)
                nc.vector.tensor_copy(tr[:], t32[:])
                return tr

            sela = const_tile(selaD[:], "sela")
            selc = const_tile(selcD[:], "selc")
            xrw = const_tile(xrwD[:], "xrw")
            fin = const_tile(finD[:], "fin")
            rw = [const_tile(rwD[k], f"rw{k}") for k in range(3)]
            w2_sb = const_tile(w2D[:], "w2")
            w3_sb = const_tile(w3D[:], "w3")

            def finish(fin):
                """Deferred chunk tail: E*P multiply, residual add, store.
                Emitted mid-next-chunk so these late-dependency ops do not
                head-of-line-block the DVE/Pool FIFOs."""
                ept_c = sp.tile([4, nb], f32, tag="ept_c", bufs=3)
                nc.vector.tensor_tensor(ept_c[:], fin["p_E"][0:4], fin["pex"][:],
                                        op=OP.mult)
                o_c = sp.tile([4, nb], f32, tag="o_c", bufs=3)
                nc.gpsimd.tensor_tensor(o_c[:], ept_c[:], fin["cen"][:], op=OP.add)
                nc.sync.dma_start(out=outs_d[fin["g"]][4 * fin["cg"]:4 * fin["cg"] + 4],
                                  in_=o_c[:])

            def stage1(g, cg, xc4, pending):
                """Gathers + evacuations + products (+ xm gather/evac) for one chunk."""
                xre = xc4[:, cg * nb2:cg * nb2 + nb]
                xim = xc4[:, cg * nb2 + nb:cg * nb2 + nb2]
                pk = []
                for k, (r0, rk) in enumerate(KSPLITS):
                    a_sl = sela[:, r0:r0 + rk]
                    c_sl = selc[:, r0:r0 + rk]
                    p_c = gp.tile([128, nb2], f32, tag="pg")
                    p_a = gp.tile([128, nb2], f32, tag="pg")
                    nc.tensor.matmul(p_c[:rk, 0:nb], c_sl, xre, start=True, stop=True)
                    nc.tensor.matmul(p_c[:rk, nb:nb2], c_sl, xim, start=True, stop=True)
                    nc.tensor.matmul(p_a[:rk, 0:nb], a_sl, xre, start=True, stop=True)
                    nc.tensor.matmul(p_a[:rk, nb:nb2], a_sl, xim, start=True, stop=True)
                    acat = ep_.tile([128, nb2], bf16, tag="acat", bufs=4)
                    ccat = ep_.tile([128, nb2], bf16, tag="ccat", bufs=4)
                    nc.scalar.copy(ccat[:rk], p_c[:rk])
                    if k == 0:
                        nc.scalar.copy(acat[:rk], p_a[:rk])
                    else:
                        nc.vector.tensor_copy(acat[:rk], p_a[:rk])
                    t01 = prp.tile([128, nb2], bf16, tag="t01", bufs=7)
                    t23 = prp.tile([128, nb2], bf16, tag="t23", bufs=7)
                    nc.vector.tensor_tensor(t01[:rk], acat[:rk], ccat[:rk], op=OP.mult)
                    if k == 2:
                        nc.vector.tensor_tensor(t23[:rk, 0:nb], acat[:rk, 0:nb],
                                                ccat[:rk, nb:nb2], op=OP.mult)
                        nc.vector.tensor_tensor(t23[:rk, nb:nb2], acat[:rk, nb:nb2],
                                                ccat[:rk, 0:nb], op=OP.mult)
                    else:
                        nc.gpsimd.tensor_tensor(t23[:rk, 0:nb], acat[:rk, 0:nb],
                                                ccat[:rk, nb:nb2], op=OP.mult)
                        nc.gpsimd.tensor_tensor(t23[:rk, nb:nb2], acat[:rk, nb:nb2],
                                                ccat[:rk, 0:nb], op=OP.mult)
                    pk.append((rw[k], rk, t01, t23))
                    if k == 1 and pending is not None:
                        finish(pending)
                return {"pk": pk, "xre": xre, "xim": xim,
                        "g": g, "cg": cg, "c": g * grp + cg}

            def stage2(st):
                """R accumulation + xm gather + T products + fin + tail."""
                pk = st["pk"]
                g, cg, c = st["g"], st["cg"], st["c"]
                rcat = ttp.tile([NO, nb2], bf16, tag="rcat", bufs=3)
                for half in range(2):
                    p_rh = rp.tile([NO, nb], f32, tag="rh")
                    for k3, (rwk, rk, t01, t23) in enumerate(pk):
                        st_ = (k3 == 0)
                        sp_ = (k3 == 2)
                        wr_ = rwk[:rk, 0 * NO:1 * NO]
                        wi_ = rwk[:rk, 1 * NO:2 * NO]
                        wrn = rwk[:rk, 2 * NO:3 * NO]
                        win = rwk[:rk, 3 * NO:4 * NO]
                        if half == 0:
                            nc.tensor.matmul(p_rh[:], wr_, t01[:rk, 0:nb], start=st_, stop=False)
                            nc.tensor.matmul(p_rh[:], wr_, t01[:rk, nb:nb2], start=False, stop=False)
                            nc.tensor.matmul(p_rh[:], wi_, t23[:rk, 0:nb], start=False, stop=False)
                            nc.tensor.matmul(p_rh[:], win, t23[:rk, nb:nb2], start=False, stop=sp_)
                        else:
                            nc.tensor.matmul(p_rh[:], wi_, t01[:rk, 0:nb], start=st_, stop=False)
                            nc.tensor.matmul(p_rh[:], wi_, t01[:rk, nb:nb2], start=False, stop=False)
                            nc.tensor.matmul(p_rh[:], wrn, t23[:rk, 0:nb], start=False, stop=False)
                            nc.tensor.matmul(p_rh[:], wr_, t23[:rk, nb:nb2], start=False, stop=sp_)
                    nc.scalar.copy(rcat[:, half * nb:half * nb + nb], p_rh[:])
                p_xm = gp.tile([128, nb2], f32, tag="pg")
                nc.tensor.matmul(p_xm[:NO, 0:nb], xrw[:], st["xre"], start=True, stop=True)
                nc.tensor.matmul(p_xm[:NO, nb:nb2], xrw[:], st["xim"], start=True, stop=True)
                xmcat = ttp.tile([NO, nb2], bf16, tag="xmcat", bufs=3)
                nc.vector.tensor_copy(xmcat[:], p_xm[:NO])
                u01 = ttp.tile([NO, nb2], bf16, tag="u01", bufs=3)
                u23 = ttp.tile([NO, nb2], bf16, tag="u23", bufs=3)
                nc.vector.tensor_tensor(u01[:], xmcat[:], rcat[:], op=OP.mult)
                nc.vector.tensor_tensor(u23[:, 0:nb], xmcat[:, 0:nb],
                                        rcat[:, nb:nb2], op=OP.mult)
                nc.gpsimd.tensor_tensor(u23[:, nb:nb2], xmcat[:, nb:nb2],
                                        rcat[:, 0:nb], op=OP.mult)
                p_h1 = tp_.tile([40, nb], f32, tag="hx")
                nc.tensor.matmul(p_h1[0:32], fin[:, 0:32], u01[:, 0:nb], start=True, stop=False)
                nc.tensor.matmul(p_h1[0:32], fin[:, 32:64], u01[:, nb:nb2], start=False, stop=False)
                nc.tensor.matmul(p_h1[0:32], fin[:, 64:96], u23[:, 0:nb], start=False, stop=False)
                nc.tensor.matmul(p_h1[0:32], fin[:, 96:128], u23[:, nb:nb2], start=False, stop=True)
                h1s = sp.tile([8, nb], bf16, tag="h1s", bufs=3)
                lrelu(h1s[:], p_h1[0:8])
                p_h2 = tp_.tile([40, nb], f32, tag="hx")
                nc.tensor.matmul(p_h2[:], w2_sb[0:8], h1s[:], start=True, stop=True)
                h2s = sp.tile([40, nb], bf16, tag="h2s", bufs=3)
                lrelu(h2s[:], p_h2[:])
                p_E = tp_.tile([40, nb], f32, tag="hx")
                nc.tensor.matmul(p_E[0:4], w3_sb[:], h2s[:], start=True, stop=True)
                cen_c = sp.tile([4, nb], f32, tag="cen_c", bufs=3)
                tsk_c = sp.tile([4, nb], f32, tag="tsk_c", bufs=3)
                nc.sync.dma_start(out=cen_c[:], in_=xcen[c, 0:4])
                nc.sync.dma_start(out=tsk_c[:], in_=xcen[c, 4:8])
                pex_c = sp.tile([4, nb], f32, tag="pex_c", bufs=3)
                nc.scalar.activation(pex_c[:], tsk_c[:], AF.Exp,
                                     scale=float(np.log(10.0) / 10.0))
                return {"p_E": p_E, "pex": pex_c, "cen": cen_c, "g": g, "cg": cg}

            pending = None
            for g in range(ngrp):
                xc4 = xp.tile([ROWS, grp * nb2], bf16, tag="xc4", bufs=2)
                nc.sync.dma_start(out=xc4[:], in_=xTb[g])
                for cg in range(grp):
                    pending = stage2(stage1(g, cg, xc4, pending))
            finish(pending)
    nc.compile()
    return nc


def _prep_core_inputs(inputs, static, folded):
    """Shard + lay out inputs per core. Returns list of in_maps."""
    import ml_dtypes
    bf16 = ml_dtypes.bfloat16
    xr = np.ascontiguousarray(inputs["x_real"]).reshape(BATCH, ROWS)
    xi = np.ascontiguousarray(inputs["x_imag"]).reshape(BATCH, ROWS)
    t0 = np.ascontiguousarray(inputs["task_info"][:, 0])
    shared = {
        "SELA": static["SELA"], "SELC": static["SELC"], "XREPW": static["XREPW"],
        "FINC": static["FINC"], "RW2": folded["RW2"], "W2L": folded["W2L"],
        "W3L": folded["W3L"],
    }
    in_maps = []
    for cc in range(NCORES):
        s = slice(cc * BCORE, (cc + 1) * BCORE)
        m = dict(shared)
        # xTb [nchunk, ROWS, 2*NB] bf16: [c, r, 0:NB]=re, [c, r, NB:]=im
        xrc = xr[s].reshape(NCHUNK, NB, ROWS).transpose(0, 2, 1)
        xic = xi[s].reshape(NCHUNK, NB, ROWS).transpose(0, 2, 1)
        xtb = np.empty((NCHUNK, ROWS, NB2), bf16)
        xtb[:, :, 0:NB] = xrc.astype(bf16)
        xtb[:, :, NB:NB2] = xic.astype(bf16)
        m["xTb"] = xtb
        # xcen [nchunk, 8, NB] f32: rows 0-3 center [re_p0, re_p1, im_p0, im_p1],
        # rows 4-7 task t0 replicated
        cen = np.empty((NCHUNK, 8, NB), np.float32)
        cen[:, 0] = xrc[:, 2 * LH + 0]
        cen[:, 1] = xrc[:, 2 * LH + 1]
        cen[:, 2] = xic[:, 2 * LH + 0]
        cen[:, 3] = xic[:, 2 * LH + 1]
        cen[:, 4:8] = t0[s].reshape(NCHUNK, 1, NB)
        m["xcen"] = cen
        in_maps.append(m)
    return in_maps


_CACHE = {}


def kernel(**inputs):
    from concourse.bass_utils import run_bass_kernel_spmd

    static = build_static()
    folded = fold_weights(
        np.asarray(inputs["W1_real"]), np.asarray(inputs["W1_imag"]),
        np.asarray(inputs["W2_real"]), np.asarray(inputs["W2_imag"]),
        np.asarray(inputs["W3_real"]), np.asarray(inputs["W3_imag"]),
    )
    if "nc" not in _CACHE:
        _CACHE["nc"] = build_nc()
    nc = _CACHE["nc"]
    in_maps = _prep_core_inputs(inputs, static, folded)
    res = run_bass_kernel_spmd(nc, in_maps, list(range(NCORES)))
    out = np.empty((BATCH, NM, 2), np.float32)
    for cc in range(NCORES):
        s0 = cc * BCORE
        for g in range(NGRP):
            o16 = res.results[cc][f"out{g}"]        # [16, NB]
            for cg in range(GRP):
                c = g * GRP + cg
                sl = slice(s0 + c * NB, s0 + (c + 1) * NB)
                out[sl, 0, 0] = o16[4 * cg + 0]
                out[sl, 1, 0] = o16[4 * cg + 1]
                out[sl, 0, 1] = o16[4 * cg + 2]
                out[sl, 1, 1] = o16[4 * cg + 3]
    return out


# revision 37
# speedup vs baseline: 1.1827x; 1.0020x over previous
"""EqPBCNN (perturbation-based nonlinearity compensation NN) Trainium2 Bass kernel.

Data-parallel over 8 NeuronCores: batch 65536 -> 8192 per core.

Math (per sample, per polarization p):
  A~[h]   = sum_q x[n_h,q] * conj(x[m_h+n_h,q])        (pol-summed, same for both p)
  F[h,p]  = SYM[h] * A~[h] * x[m_h,p]
  h1 = CLrelu(F @ W1^T); h2 = CLrelu(h1 @ W2^T); E = h2 @ W3^T
  out = x[center,p] + E * 10^(task0/10)/2

Kernel pipeline per chunk of nb=512 samples (taps on partitions, samples on
free dim, complex handled as [re|im] halves of the free dim, bf16 data
shipped pre-converted and chunk-contiguous from the host):
  stage1: gather MMs (PE, one-hot SEL) -> f32 psum pairs [A|C]x[re|im]
          pair evacuations (ACT/DVE)   -> bf16 SBUF acat=[ar|ai], ccat=[cr|ci]
          products (DVE/Pool bf16)     -> t01 = acat*ccat = [ar*cr | ai*ci]
                                          t23 = cross-sliced [ar*ci | ai*cr]
  stage2: R matmuls (PE)               -> Rre then Rim (1 psum bank each),
                                          accumulated straight from t01/t23 with
                                          sign-folded weights (G adds absorbed)
          xm gather + T products       -> u01/u23 = xmcat * rcat
          fin matmuls (PE)             -> h1 (T adds absorbed via +/- FIN blocks)
          per-chunk tail               -> lrelu/W2/lrelu/W3/exp/residual add

Measured: 232 us on silicon (NTFF exec_time_ns, max of 8 cores), rel err 4.6e-3.
"""
import numpy as np

# ---------------- problem constants (hardcoded; must match reference) -------
BATCH = 65536
MT, LH = 41, 20          # filter taps, half window
NM = 2                   # modes / polarizations
H1, H2 = 2, 10
SLOPE = 0.01
NCORES = 8
BCORE = BATCH // NCORES  # 8192
NB = 512                 # samples per chunk
NB2 = 2 * NB             # free dim of [re|im] stacked tiles
NCHUNK = BCORE // NB     # 16
GRP = 4                  # chunks per tail group
NGRP = NCHUNK // GRP
ROWS = MT * NM           # 82 = tap*2 + mode

_idx = [(m, n) for m in range(-LH, LH + 1) for n in range(-LH, LH + 1)
        if abs(m * n) <= LH and abs(m + n) <= LH and n >= m]
H = len(_idx)            # 175
M_ARR = np.array([t[0] for t in _idx], np.int32)
N_ARR = np.array([t[1] for t in _idx], np.int32)
A_TAP = N_ARR + LH           # source tap for En
C_TAP = M_ARR + N_ARR + LH   # source tap for Emn (conjugated side)
SYM = np.where(M_ARR != N_ARR, 2.0, 1.0).astype(np.float32)
M_VALS = sorted(set(M_ARR.tolist()))     # 25 distinct m values
NMV = len(M_VALS)
M_POS = {m: i for i, m in enumerate(M_VALS)}
NO = H1 * NMV * NM       # 100 rows of R space: (o, mi, p)
NSTACK = 2 * H           # 350 rows: (h, pol)
KSPLITS = [(0, 128), (128, 128), (256, NSTACK - 256)]   # psplits of the stacks


def _orow(o, mi, p):
    return (o * NMV + mi) * NM + p


def _hrow(p, o, comp):
    return (p * H1 + o) * 2 + comp


def _h2row(p, q, comp):
    return (p * H2 + q) * 2 + comp


def build_static():
    """Weight-independent constant matrices (stored f32, cast to bf16 on chip)."""
    # gather selections: stack row r = 2h+q reads x row 2*tap+q
    SELA = np.zeros((ROWS, NSTACK), np.float32)
    SELC = np.zeros((ROWS, NSTACK), np.float32)
    for h in range(H):
        for q in range(NM):
            r = 2 * h + q
            SELA[2 * A_TAP[h] + q, r] = 1.0
            SELC[2 * C_TAP[h] + q, r] = 1.0
    # xm gather: row (o,mi,p) reads tap m
    XREPW = np.zeros((ROWS, NO), np.float32)
    for o in range(H1):
        for mi, mv in enumerate(M_VALS):
            for p in range(NM):
                XREPW[2 * (mv + LH) + p, _orow(o, mi, p)] = 1.0
    # fin contraction [100, 32]: blocks for u01_lo, u01_hi, u23_lo, u23_hi
    #   T_re = u01_lo - u01_hi  -> comp 0 rows ;  T_im = u23_lo + u23_hi -> comp 1
    # FINC [100, 4*32]: term block j (32 cols, zero-padded so the matmul
    # initializes the full 32-row strip) for u01_lo, u01_hi, u23_lo, u23_hi
    FINC = np.zeros((NO, 128), np.float32)
    for o in range(H1):
        for mi in range(NMV):
            for p in range(NM):
                r = _orow(o, mi, p)
                FINC[r, 0 * 32 + _hrow(p, o, 0)] = 1.0
                FINC[r, 1 * 32 + _hrow(p, o, 0)] = -1.0
                FINC[r, 2 * 32 + _hrow(p, o, 1)] = 1.0
                FINC[r, 3 * 32 + _hrow(p, o, 1)] = 1.0
    return {"SELA": SELA, "SELC": SELC, "XREPW": XREPW, "FINC": FINC}


def fold_weights(W1r, W1i, W2r, W2i, W3r, W3i):
    """Runtime weight folding into matmul lhsT constants.

    RW2[k] : [128, 4*NO] psplit-k lhsT, col blocks [Wr, Wi, -Wr, -Wi]
    (rows (h,q)-interleaved; q rows share the weight = pol pre-sum).
    R accumulation (G adds absorbed):
      t01 = [ar*cr | ai*ci]   t23_a = ar*ci   t23_b = ai*cr
      G_re = t01_lo + t01_hi      G_im = t23_b - t23_a
      Rre  = Wr*G_re - Wi*G_im = Wr*t01_lo + Wr*t01_hi + Wi*t23_a - Wi*t23_b
      Rim  = Wi*G_re + Wr*G_im = Wi*t01_lo + Wi*t01_hi - Wr*t23_a + Wr*t23_b
    """
    Wr = (W1r * SYM[None, None, :]).astype(np.float32)   # [p, o, h]
    Wi = (W1i * SYM[None, None, :]).astype(np.float32)
    RWr = np.zeros((NSTACK, NO), np.float32)
    RWi = np.zeros((NSTACK, NO), np.float32)
    for h in range(H):
        mi = M_POS[M_ARR[h]]
        for p in range(NM):          # output pol (weights are per-pol)
            for q in range(NM):      # source stack row pol (pre-summed)
                r = 2 * h + q
                for o in range(H1):
                    c = _orow(o, mi, p)
                    RWr[r, c] += Wr[p, o, h]
                    RWi[r, c] += Wi[p, o, h]
    RW2 = np.zeros((3, 128, 4 * NO), np.float32)
    for k, (r0, rk) in enumerate(KSPLITS):
        RW2[k, :rk, 0 * NO:1 * NO] = RWr[r0:r0 + rk]
        RW2[k, :rk, 1 * NO:2 * NO] = RWi[r0:r0 + rk]
        RW2[k, :rk, 2 * NO:3 * NO] = -RWr[r0:r0 + rk]
        RW2[k, :rk, 3 * NO:4 * NO] = -RWi[r0:r0 + rk]
    # W2 lhsT [8, 40]
    W2L = np.zeros((8, 2 * H2 * NM), np.float32)
    for p in range(NM):
        for q in range(H2):
            for o in range(H1):
                W2L[_hrow(p, o, 0), _h2row(p, q, 0)] += W2r[p, q, o]
                W2L[_hrow(p, o, 1), _h2row(p, q, 0)] -= W2i[p, q, o]
                W2L[_hrow(p, o, 0), _h2row(p, q, 1)] += W2i[p, q, o]
                W2L[_hrow(p, o, 1), _h2row(p, q, 1)] += W2r[p, q, o]
    # replicate W2L at the 4 strip offsets (lhsT base must match rhs base)
    W2L4 = np.zeros((128, 2 * H2 * NM), np.float32)
    for cg in range(4):
        W2L4[32 * cg:32 * cg + 8] = W2L
    W2L = W2L4
    # W3 lhsT [40, 4]: out rows [re_p0, re_p1, im_p0, im_p1]; 1/NM folded
    W3L = np.zeros((2 * H2 * NM, 4), np.float32)
    s = 1.0 / NM
    for p in range(NM):
        for q in range(H2):
            W3L[_h2row(p, q, 0), 0 + p] += W3r[p, 0, q] * s
            W3L[_h2row(p, q, 1), 0 + p] -= W3i[p, 0, q] * s
            W3L[_h2row(p, q, 0), 2 + p] += W3i[p, 0, q] * s
            W3L[_h2row(p, q, 1), 2 + p] += W3r[p, 0, q] * s
    return {"RW2": RW2, "W2L": W2L, "W3L": W3L}


# ---------------------------------------------------------------------------
def build_nc(bcore=BCORE, nb=NB, lrelu_mode="act"):
    """Build the Bass program for one core processing `bcore` samples."""
    import concourse.bass as bass
    import concourse.bacc as bacc
    import concourse.mybir as mybir
    from concourse.tile import TileContext
    import bass_rust

    nchunk = bcore // nb
    assert nchunk * nb == bcore
    grp = GRP if nchunk % GRP == 0 else 1
    ngrp = nchunk // grp
    nb2 = 2 * nb
    f32 = mybir.dt.float32
    bf16 = mybir.dt.bfloat16
    AF = bass_rust.ActivationFunctionType
    OP = mybir.AluOpType

    nc = bacc.Bacc(None, target_bir_lowering=False, debug=False)
    xTb = nc.declare_dram_parameter("xTb", [ngrp, ROWS, grp * nb2], bf16, isOutput=False)
    xcen = nc.declare_dram_parameter("xcen", [nchunk, 8, nb], f32, isOutput=False)
    selaD = nc.declare_dram_parameter("SELA", [ROWS, NSTACK], f32, isOutput=False)
    selcD = nc.declare_dram_parameter("SELC", [ROWS, NSTACK], f32, isOutput=False)
    xrwD = nc.declare_dram_parameter("XREPW", [ROWS, NO], f32, isOutput=False)
    finD = nc.declare_dram_parameter("FINC", [NO, 128], f32, isOutput=False)
    rwD = nc.declare_dram_parameter("RW2", [3, 128, 4 * NO], f32, isOutput=False)
    w2D = nc.declare_dram_parameter("W2L", [128, 40], f32, isOutput=False)
    w3D = nc.declare_dram_parameter("W3L", [40, 4], f32, isOutput=False)
    outs_d = [nc.declare_dram_parameter(f"out{j}", [4 * grp, nb], f32, isOutput=True)
              for j in range(ngrp)]

    with TileContext(nc) as tc:
        with (
            tc.tile_pool(name="consts", bufs=1) as cp,
            tc.tile_pool(name="xt", bufs=3) as xp,
            tc.tile_pool(name="ev", bufs=2) as ep_,      # evacuated stacks
            tc.tile_pool(name="pr", bufs=2) as prp,      # products
            tc.tile_pool(name="tt", bufs=2) as ttp,      # T tiles
            tc.tile_pool(name="small", bufs=2) as sp,
            tc.tile_pool(name="gath", bufs=3, space="PSUM") as gp,   # gather pair psums
            tc.tile_pool(name="racc", bufs=1, space="PSUM") as rp,   # R halves
            tc.tile_pool(name="tailp", bufs=1, space="PSUM") as tp_,  # shared h1/h2
        ):
            def lrelu(dst, src):
                if lrelu_mode == "act":
                    nc.scalar.activation(dst, src, AF.Lrelu, alpha=SLOPE)
                else:
                    nc.vector.tensor_scalar_mul(dst, src, SLOPE)
                    nc.vector.tensor_tensor(dst, dst, src, op=OP.max)

            def const_tile(src_ap, name):
                t32 = cp.tile(list(src_ap.shape), f32, name=name + "_32")
                nc.gpsimd.dma_start(out=t32[:], in_=src_ap)
                tr = cp.tile(list(src_ap.shape), bf16, name=name)
                nc.vector.tensor_copy(tr[:], t32[:])
                return tr

            sela = const_tile(selaD[:], "sela")
            selc = const_tile(selcD[:], "selc")
            xrw = const_tile(xrwD[:], "xrw")
            fin = const_tile(finD[:], "fin")
            rw = [const_tile(rwD[k], f"rw{k}") for k in range(3)]
            w2_sb = const_tile(w2D[:], "w2")
            w3_sb = const_tile(w3D[:], "w3")

            def finish(fin):
                """Deferred chunk tail: W2/W3 MLP, E*P multiply, residual add,
                store. Emitted mid-next-chunk so these late-dependency ops do
                not head-of-line-block the PE/DVE/Pool FIFOs."""
                p_h2 = tp_.tile([40, nb], f32, tag="hx")
                nc.tensor.matmul(p_h2[:], w2_sb[0:8], fin["h1s"][:],
                                 start=True, stop=True)
                h2s = sp.tile([40, nb], bf16, tag="h2s", bufs=3)
                lrelu(h2s[:], p_h2[:])
                p_E = tp_.tile([40, nb], f32, tag="hx")
                nc.tensor.matmul(p_E[0:4], w3_sb[:], h2s[:], start=True, stop=True)
                ept_c = sp.tile([4, nb], f32, tag="ept_c", bufs=3)
                nc.vector.tensor_tensor(ept_c[:], p_E[0:4], fin["pex"][:],
                                        op=OP.mult)
                o_c = sp.tile([4, nb], f32, tag="o_c", bufs=3)
                nc.gpsimd.tensor_tensor(o_c[:], ept_c[:], fin["cen"][:], op=OP.add)
                nc.sync.dma_start(out=outs_d[fin["g"]][4 * fin["cg"]:4 * fin["cg"] + 4],
                                  in_=o_c[:])

            def stage1(g, cg, xc4, pending):
                """Gathers + evacuations + products (+ xm gather/evac) for one chunk."""
                xre = xc4[:, cg * nb2:cg * nb2 + nb]
                xim = xc4[:, cg * nb2 + nb:cg * nb2 + nb2]
                pk = []
                for k, (r0, rk) in enumerate(KSPLITS):
                    a_sl = sela[:, r0:r0 + rk]
                    c_sl = selc[:, r0:r0 + rk]
                    p_c = gp.tile([128, nb2], f32, tag="pg")
                    p_a = gp.tile([128, nb2], f32, tag="pg")
                    nc.tensor.matmul(p_c[:rk, 0:nb], c_sl, xre, start=True, stop=True)
                    nc.tensor.matmul(p_c[:rk, nb:nb2], c_sl, xim, start=True, stop=True)
                    nc.tensor.matmul(p_a[:rk, 0:nb], a_sl, xre, start=True, stop=True)
                    nc.tensor.matmul(p_a[:rk, nb:nb2], a_sl, xim, start=True, stop=True)
                    acat = ep_.tile([128, nb2], bf16, tag="acat", bufs=4)
                    ccat = ep_.tile([128, nb2], bf16, tag="ccat", bufs=4)
                    nc.scalar.copy(ccat[:rk], p_c[:rk])
                    if k == 0:
                        nc.scalar.copy(acat[:rk], p_a[:rk])
                    else:
                        nc.vector.tensor_copy(acat[:rk], p_a[:rk])
                    t01 = prp.tile([128, nb2], bf16, tag="t01", bufs=7)
                    t23 = prp.tile([128, nb2], bf16, tag="t23", bufs=7)
                    nc.vector.tensor_tensor(t01[:rk], acat[:rk], ccat[:rk], op=OP.mult)
                    if k == 2:
                        nc.vector.tensor_tensor(t23[:rk, 0:nb], acat[:rk, 0:nb],
                                                ccat[:rk, nb:nb2], op=OP.mult)
                        nc.vector.tensor_tensor(t23[:rk, nb:nb2], acat[:rk, nb:nb2],
                                                ccat[:rk, 0:nb], op=OP.mult)
                    else:
                        nc.gpsimd.tensor_tensor(t23[:rk, 0:nb], acat[:rk, 0:nb],
                                                ccat[:rk, nb:nb2], op=OP.mult)
                        nc.gpsimd.tensor_tensor(t23[:rk, nb:nb2], acat[:rk, nb:nb2],
                                                ccat[:rk, 0:nb], op=OP.mult)
                    pk.append((rw[k], rk, t01, t23))
                    if k == 1 and pending is not None:
                        finish(pending)
                return {"pk": pk, "xre": xre, "xim": xim,
                        "g": g, "cg": cg, "c": g * grp + cg}

            def stage2(st):
                """R accumulation + xm gather + T products + fin + tail."""
                pk = st["pk"]
                g, cg, c = st["g"], st["cg"], st["c"]
                rcat = ttp.tile([NO, nb2], bf16, tag="rcat", bufs=3)
                for half in range(2):
                    p_rh = rp.tile([NO, nb], f32, tag="rh")
                    for k3, (rwk, rk, t01, t23) in enumerate(pk):
                        st_ = (k3 == 0)
                        sp_ = (k3 == 2)
                        wr_ = rwk[:rk, 0 * NO:1 * NO]
                        wi_ = rwk[:rk, 1 * NO:2 * NO]
                        wrn = rwk[:rk, 2 * NO:3 * NO]
                        win = rwk[:rk, 3 * NO:4 * NO]
                        if half == 0:
                            nc.tensor.matmul(p_rh[:], wr_, t01[:rk, 0:nb], start=st_, stop=False)
                            nc.tensor.matmul(p_rh[:], wr_, t01[:rk, nb:nb2], start=False, stop=False)
                            nc.tensor.matmul(p_rh[:], wi_, t23[:rk, 0:nb], start=False, stop=False)
                            nc.tensor.matmul(p_rh[:], win, t23[:rk, nb:nb2], start=False, stop=sp_)
                        else:
                            nc.tensor.matmul(p_rh[:], wi_, t01[:rk, 0:nb], start=st_, stop=False)
                            nc.tensor.matmul(p_rh[:], wi_, t01[:rk, nb:nb2], start=False, stop=False)
                            nc.tensor.matmul(p_rh[:], wrn, t23[:rk, 0:nb], start=False, stop=False)
                            nc.tensor.matmul(p_rh[:], wr_, t23[:rk, nb:nb2], start=False, stop=sp_)
                    nc.scalar.copy(rcat[:, half * nb:half * nb + nb], p_rh[:])
                p_xm = gp.tile([128, nb2], f32, tag="pg")
                nc.tensor.matmul(p_xm[:NO, 0:nb], xrw[:], st["xre"], start=True, stop=True)
                nc.tensor.matmul(p_xm[:NO, nb:nb2], xrw[:], st["xim"], start=True, stop=True)
                xmcat = ttp.tile([NO, nb2], bf16, tag="xmcat", bufs=3)
                nc.vector.tensor_copy(xmcat[:], p_xm[:NO])
                u01 = ttp.tile([NO, nb2], bf16, tag="u01", bufs=3)
                u23 = ttp.tile([NO, nb2], bf16, tag="u23", bufs=3)
                nc.vector.tensor_tensor(u01[:], xmcat[:], rcat[:], op=OP.mult)
                nc.vector.tensor_tensor(u23[:, 0:nb], xmcat[:, 0:nb],
                                        rcat[:, nb:nb2], op=OP.mult)
                nc.gpsimd.tensor_tensor(u23[:, nb:nb2], xmcat[:, nb:nb2],
                                        rcat[:, 0:nb], op=OP.mult)
                p_h1 = tp_.tile([40, nb], f32, tag="hx")
                nc.tensor.matmul(p_h1[0:32], fin[:, 0:32], u01[:, 0:nb], start=True, stop=False)
                nc.tensor.matmul(p_h1[0:32], fin[:, 32:64], u01[:, nb:nb2], start=False, stop=False)
                nc.tensor.matmul(p_h1[0:32], fin[:, 64:96], u23[:, 0:nb], start=False, stop=False)
                nc.tensor.matmul(p_h1[0:32], fin[:, 96:128], u23[:, nb:nb2], start=False, stop=True)
                h1s = sp.tile([8, nb], bf16, tag="h1s", bufs=3)
                lrelu(h1s[:], p_h1[0:8])
                cen_c = sp.tile([4, nb], f32, tag="cen_c", bufs=3)
                tsk_c = sp.tile([4, nb], f32, tag="tsk_c", bufs=3)
                nc.sync.dma_start(out=cen_c[:], in_=xcen[c, 0:4])
                nc.sync.dma_start(out=tsk_c[:], in_=xcen[c, 4:8])
                pex_c = sp.tile([4, nb], f32, tag="pex_c", bufs=3)
                nc.scalar.activation(pex_c[:], tsk_c[:], AF.Exp,
                                     scale=float(np.log(10.0) / 10.0))
                return {"h1s": h1s, "pex": pex_c, "cen": cen_c, "g": g, "cg": cg}

            pending = None
            for g in range(ngrp):
                xc4 = xp.tile([ROWS, grp * nb2], bf16, tag="xc4", bufs=2)
                nc.sync.dma_start(out=xc4[:], in_=xTb[g])
                for cg in range(grp):
                    pending = stage2(stage1(g, cg, xc4, pending))
            finish(pending)
    nc.compile()
    return nc


def _prep_core_inputs(inputs, static, folded):
    """Shard + lay out inputs per core. Returns list of in_maps."""
    import ml_dtypes
    bf16 = ml_dtypes.bfloat16
    xr = np.ascontiguousarray(inputs["x_real"]).reshape(BATCH, ROWS)
    xi = np.ascontiguousarray(inputs["x_imag"]).reshape(BATCH, ROWS)
    t0 = np.ascontiguousarray(inputs["task_info"][:, 0])
    shared = {
        "SELA": static["SELA"], "SELC": static["SELC"], "XREPW": static["XREPW"],
        "FINC": static["FINC"], "RW2": folded["RW2"], "W2L": folded["W2L"],
        "W3L": folded["W3L"],
    }
    in_maps = []
    for cc in range(NCORES):
        s = slice(cc * BCORE, (cc + 1) * BCORE)
        m = dict(shared)
        # xTb [nchunk, ROWS, 2*NB] bf16: [c, r, 0:NB]=re, [c, r, NB:]=im
        xrc = xr[s].reshape(NCHUNK, NB, ROWS).transpose(0, 2, 1)
        xic = xi[s].reshape(NCHUNK, NB, ROWS).transpose(0, 2, 1)
        xtb = np.empty((NCHUNK, ROWS, NB2), bf16)
        xtb[:, :, 0:NB] = xrc.astype(bf16)
        xtb[:, :, NB:NB2] = xic.astype(bf16)
        m["xTb"] = xtb
        # xcen [nchunk, 8, NB] f32: rows 0-3 center [re_p0, re_p1, im_p0, im_p1],
        # rows 4-7 task t0 replicated
        cen = np.empty((NCHUNK, 8, NB), np.float32)
        cen[:, 0] = xrc[:, 2 * LH + 0]
        cen[:, 1] = xrc[:, 2 * LH + 1]
        cen[:, 2] = xic[:, 2 * LH + 0]
        cen[:, 3] = xic[:, 2 * LH + 1]
        cen[:, 4:8] = t0[s].reshape(NCHUNK, 1, NB)
        m["xcen"] = cen
        in_maps.append(m)
    return in_maps


_CACHE = {}


def kernel(**inputs):
    from concourse.bass_utils import run_bass_kernel_spmd

    static = build_static()
    folded = fold_weights(
        np.asarray(inputs["W1_real"]), np.asarray(inputs["W1_imag"]),
        np.asarray(inputs["W2_real"]), np.asarray(inputs["W2_imag"]),
        np.asarray(inputs["W3_real"]), np.asarray(inputs["W3_imag"]),
    )
    if "nc" not in _CACHE:
        _CACHE["nc"] = build_nc()
    nc = _CACHE["nc"]
    in_maps = _prep_core_inputs(inputs, static, folded)
    res = run_bass_kernel_spmd(nc, in_maps, list(range(NCORES)))
    out = np.empty((BATCH, NM, 2), np.float32)
    for cc in range(NCORES):
        s0 = cc * BCORE
        for g in range(NGRP):
            o16 = res.results[cc][f"out{g}"]        # [16, NB]
            for cg in range(GRP):
                c = g * GRP + cg
                sl = slice(s0 + c * NB, s0 + (c + 1) * NB)
                out[sl, 0, 0] = o16[4 * cg + 0]
                out[sl, 1, 0] = o16[4 * cg + 1]
                out[sl, 0, 1] = o16[4 * cg + 2]
                out[sl, 1, 1] = o16[4 * cg + 3]
    return out
